# revision 15
# baseline (speedup 1.0000x reference)
"""Trainium2 Bass kernel for nn_CrossAttention (B=4, N=M=2048, 8 heads x 64).

Sharding: 8 cores = batch(4) x sequence-half(2). Core c handles batch c//2,
query rows [ (c%2)*1024, (c%2+1)*1024 ). Each core needs its batch's full
context (replicated to the 2 cores of a batch pair); no cross-core
communication is required.

Per-core compute (all matmuls bf16 with f32 PSUM accumulation):
  xT   = transpose(x_shard)                       [512k, 1024i]
  ctxT = transpose(ctx)                           [512k, 2048m]
  qT   = tanh(Wq^T @ xT)                          [512(h,d), 1024i]
  kT   = tanh(Wkv_k^T @ ctxT)                     [512(h,d), 2048m]
  v    = ctx @ Wkv_v   (via lhsT=ctxT tiles)      [2048m, 512(h,d)]
  per head h, per key-tile mt (16 real + 1 null):
    simT[mt] = kT_h[:,mt]^T @ qT_h                [128m, 1024i]  (PSUM)
    expT[mt] = exp(simT * 1/8)                    bf16
    avT_h   += v65_h[mt]^T @ expT[mt]             [65, 1024]     (PSUM accum)
  (v65 = [v_h | ones]; the ones column accumulates the softmax denominator.
   The null token is key-tile 16: kT_null col0 = tanh(null_k), rest 0;
   v65_null row0 = [null_v, 1], rest 0.)
  avT_full[:,h,:] = avT_h[0:64] * (1/avT_h[64])   bf16 [64d, 8h, 1024i]
  outT = Wout^T @ avT_full + bout                 [512c, 1024i]
Softmax needs no max subtraction: q,k are tanh-bounded so |sim*scale| <= 8.

Host gathers the 8 outT shards ([512, 1024] each) and transposes into the
full [4, 2048, 512] output.
"""

import os
import sys

import numpy as np

sys.path.insert(0, "/opt/trn_rl_repo")

B, N, M = 4, 2048, 2048
DIM = 512
HEADS, DIM_HEAD = 8, 64
INNER = HEADS * DIM_HEAD
NSH = N // 2          # query rows per core
SCALE = DIM_HEAD ** -0.5
N_CORES = 8

_COMPILED = {}
LAST_EXEC_TIME_NS = None


def _build():
    import concourse.bass as bass
    import concourse.tile as tile
    from concourse import bacc, mybir
    from concourse.masks import make_identity

    F32 = mybir.dt.float32
    BF16 = mybir.dt.bfloat16
    Act = mybir.ActivationFunctionType

    nc = bacc.Bacc("TRN2", target_bir_lowering=False, debug=False,
                   num_devices=N_CORES)

    x_d = nc.dram_tensor("x", [NSH, DIM], F32, kind="ExternalInput").ap()
    ctx_d = nc.dram_tensor("ctx", [M, DIM], F32, kind="ExternalInput").ap()
    wq_d = nc.dram_tensor("wq", [DIM, INNER], F32, kind="ExternalInput").ap()
    wkv_d = nc.dram_tensor("wkv", [DIM, 2 * INNER], F32, kind="ExternalInput").ap()
    nk_d = nc.dram_tensor("nullk", [DIM_HEAD, 1], F32, kind="ExternalInput").ap()
    nv_d = nc.dram_tensor("nullv", [1, DIM_HEAD], F32, kind="ExternalInput").ap()
    wout_d = nc.dram_tensor("wout", [INNER, DIM], F32, kind="ExternalInput").ap()
    # host reshapes bout -> [128, 4] (c = co*128 + p)
    bout_d = nc.dram_tensor("bout", [128, 4], F32, kind="ExternalInput").ap()
    out_d = nc.dram_tensor("out", [DIM, NSH], F32, kind="ExternalOutput").ap()

    KO = DIM // 128            # 4 k-outer tiles of the model dim
    IT = NSH // 512            # 2 i-chunks of 512
    MT = M // 128              # 16 key tiles (+1 null)

    with tile.TileContext(nc) as tc:
        with (
            tc.tile_pool(name="persist", bufs=1) as P,
            tc.tile_pool(name="stage", bufs=3) as ST,
            tc.tile_pool(name="den", bufs=2) as STD,
            tc.tile_pool(name="exp", bufs=3) as EX,
            tc.tile_pool(name="gen_ps", bufs=2, space="PSUM") as PS,
            tc.tile_pool(name="acc_ps", bufs=2, space="PSUM") as PSA,
        ):
            ident = P.tile([128, 128], F32, tag="ident")
            make_identity(nc, ident)

            # ---- persistent SBUF tensors (bf16) ----
            xT = P.tile([128, KO, NSH], BF16, tag="xT")
            ctxT = P.tile([128, KO, M], BF16, tag="ctxT")
            wq_b = P.tile([128, KO, INNER], BF16, tag="wq")
            wkv_b = P.tile([128, KO, 2 * INNER], BF16, tag="wkv")
            wout_b = P.tile([64, HEADS, DIM], BF16, tag="wout")
            qT = P.tile([64, HEADS, NSH], BF16, tag="qT")
            kT = P.tile([64, HEADS, M], BF16, tag="kT")
            v65 = P.tile([128, MT, HEADS, 65], BF16, tag="v65")
            kT_null = P.tile([64, 128], BF16, tag="kTnull")
            v65_null = P.tile([128, 65], BF16, tag="v65null")
            avT_full = P.tile([64, HEADS, NSH], BF16, tag="avT")
            bout_sb = P.tile([128, 4], F32, tag="bout")

            # ---- weights: load f32, cast to bf16 ----
            for ko in range(KO):
                wq_st = ST.tile([128, INNER], F32, tag="wst")
                nc.sync.dma_start(wq_st[:], wq_d[ko * 128:(ko + 1) * 128, :])
                nc.vector.tensor_copy(wq_b[:, ko, :], wq_st[:])
            for ko in range(KO):
                wkv_st = ST.tile([128, 2 * INNER], F32, tag="wst2")
                nc.sync.dma_start(wkv_st[:], wkv_d[ko * 128:(ko + 1) * 128, :])
                nc.vector.tensor_copy(wkv_b[:, ko, :], wkv_st[:])
            for h in range(HEADS):
                wo_st = ST.tile([64, DIM], F32, tag="wst")
                nc.sync.dma_start(wo_st[:], wout_d[h * 64:(h + 1) * 64, :])
                nc.vector.tensor_copy(wout_b[:, h, :], wo_st[:])
            nc.sync.dma_start(bout_sb[:], bout_d[:])

            # ---- null-token constant tiles ----
            nk_st = ST.tile([DIM_HEAD, 1], F32, tag="nk")
            nc.sync.dma_start(nk_st[:], nk_d[:])
            nc.gpsimd.memset(kT_null[:], 0.0)
            nc.scalar.activation(kT_null[:, 0:1], nk_st[:], Act.Tanh)
            nv_st = ST.tile([1, DIM_HEAD], F32, tag="nv")
            nc.sync.dma_start(nv_st[:], nv_d[:])
            nc.gpsimd.memset(v65_null[:], 0.0)
            nc.vector.tensor_copy(v65_null[0:1, 0:DIM_HEAD], nv_st[:])
            nc.vector.memset(v65_null[0:1, 64:65], 1.0)
            # ones column of v65 (denominator accumulator)
            nc.vector.memset(v65[:, :, :, 64:65], 1.0)

            # ---- transpose x and ctx into [k, *] layouts (PE transpose) ----
            def load_transpose(src_ap, n_rows, dstT):
                # src [n_rows, DIM] f32 -> dstT [128, KO, n_rows] bf16
                for io in range(n_rows // 128):
                    row_st = ST.tile([128, DIM], F32, tag="rows")
                    nc.sync.dma_start(
                        row_st[:], src_ap[io * 128:(io + 1) * 128, :])
                    for ko in range(KO):
                        pst = PS.tile([128, 1024], F32, tag="ps")
                        pt = pst[:, 0:128]
                        nc.tensor.transpose(
                            pt, row_st[:, ko * 128:(ko + 1) * 128], ident[:])
                        nc.vector.tensor_copy(
                            dstT[:, ko, io * 128:(io + 1) * 128], pt)

            load_transpose(x_d, NSH, xT)
            load_transpose(ctx_d, M, ctxT)

            # ---- qT = tanh(Wq^T @ xT), per-head M=64 tiles (heads at p0-63) ----
            for h in range(HEADS):
                for ich in range(IT):
                    ps = PS.tile([128, 1024], F32, tag="ps")
                    pq = ps[0:64, 0:512]
                    for kt in range(KO):
                        nc.tensor.matmul(
                            pq,
                            lhsT=wq_b[:, kt, h * 64:(h + 1) * 64],
                            rhs=xT[:, kt, ich * 512:(ich + 1) * 512],
                            start=(kt == 0), stop=(kt == KO - 1))
                    nc.scalar.activation(
                        qT[:, h, ich * 512:(ich + 1) * 512], pq, Act.Tanh)

            # ---- kT = tanh(Wkv_k^T @ ctxT), per-head M=64 tiles ----
            for h in range(HEADS):
                for mch in range(M // 512):
                    ps = PS.tile([128, 1024], F32, tag="ps")
                    pk = ps[0:64, 0:512]
                    for kt in range(KO):
                        nc.tensor.matmul(
                            pk,
                            lhsT=wkv_b[:, kt, h * 64:(h + 1) * 64],
                            rhs=ctxT[:, kt, mch * 512:(mch + 1) * 512],
                            start=(kt == 0), stop=(kt == KO - 1))
                    nc.scalar.activation(
                        kT[:, h, mch * 512:(mch + 1) * 512], pk, Act.Tanh)

            # ---- v = ctx @ Wkv_v  (natural [m, (h,d)] layout) ----
            for mt in range(MT):
                ps = PS.tile([128, 1024], F32, tag="ps")
                pv = ps[:, 0:512]
                for kt in range(KO):
                    nc.tensor.matmul(
                        pv,
                        lhsT=ctxT[:, kt, mt * 128:(mt + 1) * 128],
                        rhs=wkv_b[:, kt, INNER:2 * INNER],
                        start=(kt == 0), stop=(kt == KO - 1))
                nc.vector.tensor_copy(
                    v65[:, mt, :, 0:DIM_HEAD],
                    pv.rearrange("p (h d) -> p h d", d=DIM_HEAD))

            # ---- attention per head ----
            for h in range(HEADS):
                avt = PSA.tile([65, 1024], F32, tag="avt")
                for mt in range(MT + 1):
                    ps = PS.tile([128, 1024], F32, tag="ps")
                    if mt < MT:
                        lhs_k = kT[:, h, mt * 128:(mt + 1) * 128]
                        lhs_v = v65[:, mt, h, :]
                    else:
                        lhs_k = kT_null[:]
                        lhs_v = v65_null[:]
                    for ich in range(IT):
                        nc.tensor.matmul(
                            ps[:, ich * 512:(ich + 1) * 512],
                            lhsT=lhs_k,
                            rhs=qT[:, h, ich * 512:(ich + 1) * 512],
                            start=True, stop=True)
                    expT = EX.tile([128, 1024], BF16, tag="expT")
                    nc.scalar.activation(expT[:], ps[:], Act.Exp, scale=SCALE)
                    for ich in range(IT):
                        nc.tensor.matmul(
                            avt[:, ich * 512:(ich + 1) * 512],
                            lhsT=lhs_v,
                            rhs=expT[:, ich * 512:(ich + 1) * 512],
                            start=(mt == 0), stop=(mt == MT))
                # normalize: avT_full[:, h, :] = avt[0:64] / avt[64]
                den = STD.tile([128, NSH], F32, tag="den")
                nc.vector.reciprocal(den[64:65, :], avt[64:65, :])
                # move the reciprocal row to partition 0, then broadcast
                den0 = STD.tile([1, NSH], F32, tag="den0")
                nc.sync.dma_start(den0[0:1, :], den[64:65, :])
                denb = STD.tile([64, NSH], F32, tag="denb")
                nc.gpsimd.partition_broadcast(denb[:], den0[0:1, :])
                nc.vector.tensor_mul(
                    avT_full[:, h, :], avt[0:64, :], denb[:])

            # ---- outT = Wout^T @ avT_full + bout; stream to DRAM ----
            outT_d = out_d.rearrange("(co p) i -> p co i", p=128)
            for ct in range(4):
                for ich in range(IT):
                    ps = PS.tile([128, 1024], F32, tag="ps")
                    pso = ps[:, 0:512]
                    for h in range(HEADS):
                        nc.tensor.matmul(
                            pso,
                            lhsT=wout_b[:, h, ct * 128:(ct + 1) * 128],
                            rhs=avT_full[:, h, ich * 512:(ich + 1) * 512],
                            start=(h == 0), stop=(h == HEADS - 1))
                    ost = ST.tile([128, 512], F32, tag="ost")
                    nc.vector.tensor_add(
                        ost[:], pso,
                        bout_sb[:, ct:ct + 1].to_broadcast((128, 512)))
                    nc.sync.dma_start(
                        outT_d[:, ct, ich * 512:(ich + 1) * 512], ost[:])

    nc.compile()
    return nc


def _get_compiled():
    if "nc" not in _COMPILED:
        _COMPILED["nc"] = _build()
    return _COMPILED["nc"]


def kernel(x, context, Wq, Wkv, null_k, null_v, Wout, bout):
    global LAST_EXEC_TIME_NS
    from concourse.bass_utils import run_bass_kernel_spmd

    x = np.ascontiguousarray(np.asarray(x, dtype=np.float32))
    context = np.ascontiguousarray(np.asarray(context, dtype=np.float32))
    nk = np.asarray(null_k, np.float32).reshape(64, 1).copy()
    nv = np.asarray(null_v, np.float32).reshape(1, 64)
    bout_r = np.asarray(bout, np.float32).reshape(4, 128).T.copy()
    wq = np.ascontiguousarray(np.asarray(Wq, np.float32))
    wkv = np.ascontiguousarray(np.asarray(Wkv, np.float32))
    wout = np.ascontiguousarray(np.asarray(Wout, np.float32))

    in_maps = []
    for c in range(N_CORES):
        b, j = c // 2, c % 2
        in_maps.append({
            "x": np.ascontiguousarray(x[b, j * NSH:(j + 1) * NSH, :]),
            "ctx": np.ascontiguousarray(context[b]),
            "wq": wq,
            "wkv": wkv,
            "nullk": nk,
            "nullv": nv,
            "wout": wout,
            "bout": bout_r,
        })

    nc = _get_compiled()
    res = run_bass_kernel_spmd(nc, in_maps, core_ids=list(range(N_CORES)))
    LAST_EXEC_TIME_NS = res.exec_time_ns

    out = np.empty((B, N, DIM), np.float32)
    for c in range(N_CORES):
        b, j = c // 2, c % 2
        out[b, j * NSH:(j + 1) * NSH, :] = res.results[c]["out"].T
    return out


# revision 19
# speedup vs baseline: 1.1092x; 1.1092x over previous
"""Trainium2 Bass kernel for nn_CrossAttention (B=4, N=M=2048, 8 heads x 64).

Sharding: 8 cores = batch(4) x sequence-half(2). Core c handles batch c//2,
query rows [ (c%2)*1024, (c%2+1)*1024 ). Each core needs its batch's full
context (replicated to the 2 cores of a batch pair); no cross-core
communication is required.

Per-core compute (all matmuls bf16 with f32 PSUM accumulation):
  xT   = transpose(x_shard)                       [512k, 1024i]
  ctxT = transpose(ctx)                           [512k, 2048m]
  qT   = tanh(Wq^T @ xT)                          [512(h,d), 1024i]
  kT   = tanh(Wkv_k^T @ ctxT)                     [512(h,d), 2048m]
  v    = ctx @ Wkv_v   (via lhsT=ctxT tiles)      [2048m, 512(h,d)]
  per head h, per key-tile mt (16 real + 1 null):
    simT[mt] = kT_h[:,mt]^T @ qT_h                [128m, 1024i]  (PSUM)
    expT[mt] = exp(simT * 1/8)                    bf16
    avT_h   += v65_h[mt]^T @ expT[mt]             [65, 1024]     (PSUM accum)
  (v65 = [v_h | ones]; the ones column accumulates the softmax denominator.
   The null token is key-tile 16: kT_null col0 = tanh(null_k), rest 0;
   v65_null row0 = [null_v, 1], rest 0.)
  avT_full[:,h,:] = avT_h[0:64] * (1/avT_h[64])   bf16 [64d, 8h, 1024i]
  outT = Wout^T @ avT_full + bout                 [512c, 1024i]
Softmax needs no max subtraction: q,k are tanh-bounded so |sim*scale| <= 8.

Host gathers the 8 outT shards ([512, 1024] each) and transposes into the
full [4, 2048, 512] output.
"""

import os
import sys

import numpy as np

sys.path.insert(0, "/opt/trn_rl_repo")

B, N, M = 4, 2048, 2048
DIM = 512
HEADS, DIM_HEAD = 8, 64
INNER = HEADS * DIM_HEAD
NSH = N // 2          # query rows per core
SCALE = DIM_HEAD ** -0.5
N_CORES = 8

_COMPILED = {}
LAST_EXEC_TIME_NS = None


def _build():
    import concourse.bass as bass
    import concourse.tile as tile
    from concourse import bacc, mybir
    from concourse.masks import make_identity

    F32 = mybir.dt.float32
    BF16 = mybir.dt.bfloat16
    Act = mybir.ActivationFunctionType

    nc = bacc.Bacc("TRN2", target_bir_lowering=False, debug=False,
                   num_devices=N_CORES)

    x_d = nc.dram_tensor("x", [NSH, DIM], F32, kind="ExternalInput").ap()
    ctx_d = nc.dram_tensor("ctx", [M, DIM], F32, kind="ExternalInput").ap()
    wq_d = nc.dram_tensor("wq", [DIM, INNER], F32, kind="ExternalInput").ap()
    wkv_d = nc.dram_tensor("wkv", [DIM, 2 * INNER], F32, kind="ExternalInput").ap()
    nk_d = nc.dram_tensor("nullk", [DIM_HEAD, 1], F32, kind="ExternalInput").ap()
    nv_d = nc.dram_tensor("nullv", [1, DIM_HEAD], F32, kind="ExternalInput").ap()
    wout_d = nc.dram_tensor("wout", [INNER, DIM], F32, kind="ExternalInput").ap()
    # host reshapes bout -> [128, 4] (c = co*128 + p)
    bout_d = nc.dram_tensor("bout", [128, 4], F32, kind="ExternalInput").ap()
    out_d = nc.dram_tensor("out", [DIM, NSH], F32, kind="ExternalOutput").ap()

    KO = DIM // 128            # 4 k-outer tiles of the model dim
    IT = NSH // 512            # 2 i-chunks of 512
    MT = M // 128              # 16 key tiles (+1 null)

    with tile.TileContext(nc) as tc:
        with (
            tc.tile_pool(name="persist", bufs=1) as P,
            tc.tile_pool(name="stage", bufs=3) as ST,
            tc.tile_pool(name="den", bufs=1) as STD,
            tc.tile_pool(name="exp", bufs=3) as EX,
            tc.tile_pool(name="gen_ps", bufs=2, space="PSUM") as PS,
            tc.tile_pool(name="acc_ps", bufs=2, space="PSUM") as PSA,
        ):
            ident = P.tile([128, 128], BF16, tag="ident")
            make_identity(nc, ident)

            # ---- persistent SBUF tensors (bf16) ----
            xT = P.tile([128, KO, NSH], BF16, tag="xT")
            ctxT = P.tile([128, KO, M], BF16, tag="ctxT")
            wq_b = P.tile([128, KO, INNER], BF16, tag="wq")
            wkv_b = P.tile([128, KO, 2 * INNER], BF16, tag="wkv")
            wout_b = P.tile([64, HEADS, DIM], BF16, tag="wout")
            qT = P.tile([64, HEADS, NSH], BF16, tag="qT")
            kT = P.tile([64, HEADS, M], BF16, tag="kT")
            v65 = P.tile([128, MT, HEADS, 65], BF16, tag="v65")
            kT_null = P.tile([64, 128], BF16, tag="kTnull")
            v65_null = P.tile([128, 65], BF16, tag="v65null")
            avT_full = P.tile([64, HEADS, NSH], BF16, tag="avT")
            bout_sb = P.tile([128, 4], F32, tag="bout")

            # ---- weights: load f32, cast to bf16 ----
            for ko in range(KO):
                wq_st = ST.tile([128, INNER], F32, tag="wst")
                nc.sync.dma_start(wq_st[:], wq_d[ko * 128:(ko + 1) * 128, :])
                nc.vector.tensor_copy(wq_b[:, ko, :], wq_st[:])
            for ko in range(KO):
                wkv_st = ST.tile([128, 2 * INNER], F32, tag="wst2")
                nc.sync.dma_start(wkv_st[:], wkv_d[ko * 128:(ko + 1) * 128, :])
                nc.vector.tensor_copy(wkv_b[:, ko, :], wkv_st[:])
            for h in range(HEADS):
                wo_st = ST.tile([64, DIM], F32, tag="wst")
                nc.sync.dma_start(wo_st[:], wout_d[h * 64:(h + 1) * 64, :])
                nc.vector.tensor_copy(wout_b[:, h, :], wo_st[:])
            nc.sync.dma_start(bout_sb[:], bout_d[:])

            # ---- null-token constant tiles ----
            nk_st = ST.tile([DIM_HEAD, 1], F32, tag="nk")
            nc.sync.dma_start(nk_st[:], nk_d[:])
            nc.gpsimd.memset(kT_null[:], 0.0)
            nc.scalar.activation(kT_null[:, 0:1], nk_st[:], Act.Tanh)
            nv_st = ST.tile([1, DIM_HEAD], F32, tag="nv")
            nc.sync.dma_start(nv_st[:], nv_d[:])
            nc.gpsimd.memset(v65_null[:], 0.0)
            nc.vector.tensor_copy(v65_null[0:1, 0:DIM_HEAD], nv_st[:])
            nc.vector.memset(v65_null[0:1, 64:65], 1.0)
            # ones column of v65 (denominator accumulator)
            nc.vector.memset(v65[:, :, :, 64:65], 1.0)

            # ---- transpose x and ctx into [k, *] layouts (PE transpose) ----
            def load_transpose(src_ap, n_rows, dstT):
                # src [n_rows, DIM] f32 -> dstT [128, KO, n_rows] bf16
                for io in range(n_rows // 128):
                    row_st = ST.tile([128, DIM], F32, tag="rows")
                    nc.sync.dma_start(
                        row_st[:], src_ap[io * 128:(io + 1) * 128, :])
                    row_bf = ST.tile([128, DIM], BF16, tag="rowsbf")
                    nc.vector.tensor_copy(row_bf[:], row_st[:])
                    for ko in range(KO):
                        pst = PS.tile([128, 128], BF16, tag="ps")
                        nc.tensor.transpose(
                            pst[:], row_bf[:, ko * 128:(ko + 1) * 128],
                            ident[:])
                        nc.vector.tensor_copy(
                            dstT[:, ko, io * 128:(io + 1) * 128], pst[:])

            load_transpose(x_d, NSH, xT)
            load_transpose(ctx_d, M, ctxT)

            # ---- q/k projections at M=128 (2 heads per tile), tanh, then
            # redistribute the two 64-row head halves to partitions 0-63
            # via SBUF->SBUF DMA (engines can't cross partitions; DMA can).
            def proj_pair(w_sb, w_off, src_T, n_cols, dstT, jt):
                # computes tanh(W[:, 128 cols of head pair jt]^T @ src_T)
                # -> dstT[:, 2jt, :], dstT[:, 2jt+1, :]
                for cch in range(n_cols // 1024):
                    ps = PS.tile([128, 1024], F32, tag="ps")
                    for half in range(2):
                        for kt in range(KO):
                            nc.tensor.matmul(
                                ps[:, half * 512:(half + 1) * 512],
                                lhsT=w_sb[:, kt,
                                          w_off + jt * 128:
                                          w_off + (jt + 1) * 128],
                                rhs=src_T[:, kt,
                                          cch * 1024 + half * 512:
                                          cch * 1024 + (half + 1) * 512],
                                start=(kt == 0), stop=(kt == KO - 1))
                    pb = ST.tile([128, 1024], BF16, tag="projst")
                    nc.scalar.activation(pb[:], ps[:], Act.Tanh)
                    for half in range(2):
                        nc.sync.dma_start(
                            dstT[:, 2 * jt + half,
                                 cch * 1024:(cch + 1) * 1024],
                            pb[half * 64:half * 64 + 64, :])

            for jt in range(4):
                proj_pair(wq_b, 0, xT, NSH, qT, jt)
            for jt in range(4):
                proj_pair(wkv_b, 0, ctxT, M, kT, jt)

            # ---- v = ctx @ Wkv_v  (natural [m, (h,d)] layout) ----
            for mt in range(MT):
                ps = PS.tile([128, 1024], F32, tag="ps")
                pv = ps[:, 0:512]
                for kt in range(KO):
                    nc.tensor.matmul(
                        pv,
                        lhsT=ctxT[:, kt, mt * 128:(mt + 1) * 128],
                        rhs=wkv_b[:, kt, INNER:2 * INNER],
                        start=(kt == 0), stop=(kt == KO - 1))
                nc.vector.tensor_copy(
                    v65[:, mt, :, 0:DIM_HEAD],
                    pv.rearrange("p (h d) -> p h d", d=DIM_HEAD))

            # ---- attention per head ----
            for h in range(HEADS):
                avt = PSA.tile([65, 1024], F32, tag="avt")
                for mt in range(MT + 1):
                    ps = PS.tile([128, 1024], F32, tag="ps")
                    if mt < MT:
                        lhs_k = kT[:, h, mt * 128:(mt + 1) * 128]
                        lhs_v = v65[:, mt, h, :]
                    else:
                        lhs_k = kT_null[:]
                        lhs_v = v65_null[:]
                    for ich in range(IT):
                        nc.tensor.matmul(
                            ps[:, ich * 512:(ich + 1) * 512],
                            lhsT=lhs_k,
                            rhs=qT[:, h, ich * 512:(ich + 1) * 512],
                            start=True, stop=True)
                    expT = EX.tile([128, 1024], BF16, tag="expT")
                    nc.scalar.activation(expT[:], ps[:], Act.Exp, scale=SCALE)
                    for ich in range(IT):
                        nc.tensor.matmul(
                            avt[:, ich * 512:(ich + 1) * 512],
                            lhsT=lhs_v,
                            rhs=expT[:, ich * 512:(ich + 1) * 512],
                            start=(mt == 0), stop=(mt == MT))
                # normalize: avT_full[:, h, :] = avt[0:64] / avt[64]
                den = STD.tile([128, NSH], F32, tag="den")
                nc.vector.reciprocal(den[64:65, :], avt[64:65, :])
                # move the reciprocal row to partition 0, then broadcast
                den0 = STD.tile([1, NSH], F32, tag="den0")
                nc.sync.dma_start(den0[0:1, :], den[64:65, :])
                denb = STD.tile([64, NSH], F32, tag="denb")
                nc.gpsimd.partition_broadcast(denb[:], den0[0:1, :])
                nc.vector.tensor_mul(
                    avT_full[:, h, :], avt[0:64, :], denb[:])

            # ---- outT = Wout^T @ avT_full + bout; stream to DRAM ----
            outT_d = out_d.rearrange("(co p) i -> p co i", p=128)
            for ct in range(4):
                for ich in range(IT):
                    ps = PS.tile([128, 1024], F32, tag="ps")
                    pso = ps[:, 0:512]
                    for h in range(HEADS):
                        nc.tensor.matmul(
                            pso,
                            lhsT=wout_b[:, h, ct * 128:(ct + 1) * 128],
                            rhs=avT_full[:, h, ich * 512:(ich + 1) * 512],
                            start=(h == 0), stop=(h == HEADS - 1))
                    ost = ST.tile([128, 512], F32, tag="ost")
                    nc.vector.tensor_add(
                        ost[:], pso,
                        bout_sb[:, ct:ct + 1].to_broadcast((128, 512)))
                    nc.sync.dma_start(
                        outT_d[:, ct, ich * 512:(ich + 1) * 512], ost[:])

    nc.compile()
    return nc


def _get_compiled():
    if "nc" not in _COMPILED:
        _COMPILED["nc"] = _build()
    return _COMPILED["nc"]


def kernel(x, context, Wq, Wkv, null_k, null_v, Wout, bout):
    global LAST_EXEC_TIME_NS
    from concourse.bass_utils import run_bass_kernel_spmd

    x = np.ascontiguousarray(np.asarray(x, dtype=np.float32))
    context = np.ascontiguousarray(np.asarray(context, dtype=np.float32))
    nk = np.asarray(null_k, np.float32).reshape(64, 1).copy()
    nv = np.asarray(null_v, np.float32).reshape(1, 64)
    bout_r = np.asarray(bout, np.float32).reshape(4, 128).T.copy()
    wq = np.ascontiguousarray(np.asarray(Wq, np.float32))
    wkv = np.ascontiguousarray(np.asarray(Wkv, np.float32))
    wout = np.ascontiguousarray(np.asarray(Wout, np.float32))

    in_maps = []
    for c in range(N_CORES):
        b, j = c // 2, c % 2
        in_maps.append({
            "x": np.ascontiguousarray(x[b, j * NSH:(j + 1) * NSH, :]),
            "ctx": np.ascontiguousarray(context[b]),
            "wq": wq,
            "wkv": wkv,
            "nullk": nk,
            "nullv": nv,
            "wout": wout,
            "bout": bout_r,
        })

    nc = _get_compiled()
    res = run_bass_kernel_spmd(nc, in_maps, core_ids=list(range(N_CORES)))
    LAST_EXEC_TIME_NS = res.exec_time_ns

    out = np.empty((B, N, DIM), np.float32)
    for c in range(N_CORES):
        b, j = c // 2, c % 2
        out[b, j * NSH:(j + 1) * NSH, :] = res.results[c]["out"].T
    return out
